# revision 8
# baseline (speedup 1.0000x reference)
"""Trainium2 Bass kernel for a dense transformer block (B=64,T=320,E=1024,H=16).

Strategy: pure data-parallel over batch across 8 NeuronCores (8 batches/core,
no collectives).  Per core, one Bass program runs the whole block in phases:

  A: LN1 (token-major) -> transpose -> zT (feature-major, SBUF-resident)
     -> QKV matmuls (q,k feature-major to HBM; v token-major to HBM)
  B: per (batch, head): scoresT = k^T.T @ q^T  (s on partitions, t on free),
     causal mask on diagonal blocks, exp (no max-sub needed: LN bounds scores),
     column sums via ones-matmul on PE, 1/sum broadcast via 0-stride DMA,
     attn^T = v.T @ weiT accumulated over s-chunks, normalized on copy-out.
  C: proj token-major (attnT chunks stationary, Wo moving) + residual + LN2
     -> transpose -> z2T (feature-major) to HBM
  D/E: fc1 (W1 stationary, z2T moving) -> relu+bias -> a1T (SBUF) ->
     fc2 (W2 stationary, a1T moving) -> +b2 -> transpose -> +x2 -> out

All matmul operands are fp16 (PE runs fp16 at full 1 cyc/row with FWL);
accumulation is fp32 in PSUM; the residual stream stays fp32 end-to-end.
LayerNorm scale/bias are folded into the adjacent weights host-side (exact).
"""

import sys

for _p in ("/opt/trn_rl_repo", "/opt/pypackages"):
    if _p not in sys.path:
        sys.path.append(_p)

import numpy as np

import concourse.bass as bass
import concourse.mybir as mybir
import concourse.tile as tile
import bass_rust

B, T, E, H = 64, 320, 1024, 16
HS = E // H  # 64
FF = 4 * E  # 4096
NCORES = 8
B_LOC = B // NCORES  # 8
EPS = 1e-5
F16 = mybir.dt.float16
F32 = mybir.dt.float32

MAX_DRAIN_WAITS = 1  # this walrus allows 1 sync wait per TPB_CTRL instruction


class SplitDrainTileContext(tile.TileContext):
    """TileContext whose exit drain spreads its sem waits over NOP carriers
    (the toolchain's TPB_CTRL lowering rejects >1 sync wait on one inst)."""

    WAIT_LIMIT = 1

    def _add_instruction(self, inst):
        si = getattr(inst, "sync_info", None)
        lim = self.WAIT_LIMIT
        if si is not None and si.on_wait and len(si.on_wait) > lim:
            waits = list(si.on_wait)
            extra, keep = waits[:-lim], waits[-lim:]
            inst.sync_info = mybir.SyncInfo(
                on_wait=keep, on_update=list(si.on_update or [])
            )
            for i in range(0, len(extra), lim):
                nop = mybir.InstNoOp(
                    name=self.nc.get_next_instruction_name(), ins=[], outs=[]
                )
                nop.engine = inst.engine
                nop.sync_info = mybir.SyncInfo(
                    on_wait=extra[i : i + lim], on_update=[]
                )
                super()._add_instruction(nop)
        super()._add_instruction(inst)

    def _drain_and_barrier(self, tick_clock, wait_clock):
        nc = self.nc
        carriers = [nc.sync.nop(nofuse=True) for _ in range(64)]
        drain_inst = nc.sync.drain()
        wait_clock.add_sem_waits(
            drain_inst.ins, bass_rust.ScopedClock({None: tick_clock.global_clock})
        )
        si = drain_inst.ins.sync_info
        waits = list(si.on_wait or [])
        drain_inst.ins.sync_info = mybir.SyncInfo(
            on_wait=[], on_update=list(si.on_update or [])
        )
        ci = 0
        for i in range(0, len(waits), MAX_DRAIN_WAITS):
            chunk = waits[i : i + MAX_DRAIN_WAITS]
            carriers[ci].ins.sync_info = mybir.SyncInfo(on_wait=chunk, on_update=[])
            ci += 1
        nc.all_engine_barrier()
        assert self.sems is not None
        popped = nc._tile_sem_poison_stack.pop()
        assert popped is self._sem_poison
        nc.clear_and_free_semaphores(list(self.sems.allocated().values()))
        nc.all_engine_barrier()


def _token_tiles(n, w):
    return [(o, min(w, n - o)) for o in range(0, n, w)]


def build_program(b_loc=B_LOC):
    """Build the per-core Bass program for b_loc batches (b_loc*T tokens)."""
    ntok = b_loc * T
    nc = bass.Bass()

    x_d = nc.dram_tensor("x", [ntok, E], F32, kind="ExternalInput")
    wqk_d = nc.dram_tensor("wqk", [128, 8, 16, 128], F16, kind="ExternalInput")
    wv_d = nc.dram_tensor("wv", [128, 8, E], F16, kind="ExternalInput")
    wo_d = nc.dram_tensor("wo", [128, 8, E], F16, kind="ExternalInput")
    w1_d = nc.dram_tensor("w1", [128, 8, 32, 128], F16, kind="ExternalInput")
    w2_d = nc.dram_tensor("w2", [128, 32, 8, 128], F16, kind="ExternalInput")
    bo_b_d = nc.dram_tensor("bo_bcast", [128, E], F32, kind="ExternalInput")
    b1t_d = nc.dram_tensor("b1t", [128, 32], F32, kind="ExternalInput")
    b2t_d = nc.dram_tensor("b2t", [128, 8], F32, kind="ExternalInput")
    maskt_d = nc.dram_tensor("maskT", [128, 128], F32, kind="ExternalInput")
    ident_d = nc.dram_tensor("ident16", [128, 128], F16, kind="ExternalInput")
    out_d = nc.dram_tensor("out", [ntok, E], F32, kind="ExternalOutput")

    Identity = mybir.ActivationFunctionType.Identity
    Exp = mybir.ActivationFunctionType.Exp
    Relu = mybir.ActivationFunctionType.Relu
    Sqrt = mybir.ActivationFunctionType.Sqrt

    with SplitDrainTileContext(nc) as tc:
        ctx_consts = tc.tile_pool(name="consts", bufs=1)
        ctx_dram = tc.tile_pool(name="dram", bufs=1, space="DRAM")
        with ctx_consts as consts, ctx_dram as dram:
            # --- constants ---
            bo_b = consts.tile([128, E], F32)
            nc.sync.dma_start(out=bo_b, in_=bo_b_d[:, :])
            csml = consts.tile([128, 41 + 32], F32)
            b1t = csml[:, 0:32]
            nc.sync.dma_start(out=b1t, in_=b1t_d[:, :])
            b2t = csml[:, 32:40]
            nc.sync.dma_start(out=b2t, in_=b2t_d[:, :])
            eps_t = csml[:, 40:41]
            nc.vector.memset(eps_t, EPS)
            ones16 = csml[:, 41:42].bitcast(F16)[:, 0:1]
            nc.vector.memset(ones16, 1.0)
            maskT = consts.tile([128, 128], F32)
            nc.sync.dma_start(out=maskT, in_=maskt_d[:, :])
            ident = consts.tile([128, 128], F16)
            nc.sync.dma_start(out=ident, in_=ident_d[:, :])

            # --- DRAM scratch (Tile-tracked) ---
            qT_s = dram.tile([128, 8, ntok], F16)
            kT_s = dram.tile([128, 8, ntok], F16)
            v_s = dram.tile([ntok, H * 65], F16)
            aT_s = dram.tile([128, 8, ntok], F16)
            x2_s = dram.tile([ntok, E], F32)
            z2T_s = dram.tile([128, 8, ntok], F16)

            def layer_norm(pool, xt, tw, z_out):
                """token-major LN of xt[:tw, :E] -> z_out[:tw, :E] (fp16)."""
                stats = pool.tile([128, 2, 6], F32, tag="ln_stats")
                nc.vector.bn_stats(out=stats[:tw, 0, :], in_=xt[:tw, 0:512])
                nc.vector.bn_stats(out=stats[:tw, 1, :], in_=xt[:tw, 512:1024])
                mv = pool.tile([128, 2], F32, tag="ln_mv")
                nc.vector.bn_aggr(out=mv[:tw], in_=stats[:tw])
                r = pool.tile([128, 1], F32, tag="ln_r")
                nc.scalar.activation(
                    out=r[:tw], in_=mv[:tw, 1:2], func=Sqrt, bias=eps_t[:tw], scale=1.0
                )
                nc.vector.reciprocal(out=r[:tw], in_=r[:tw])
                nmr = pool.tile([128, 1], F32, tag="ln_nmr")
                nc.vector.tensor_mul(nmr[:tw], mv[:tw, 0:1], r[:tw])
                nc.vector.tensor_scalar_mul(nmr[:tw], nmr[:tw], -1.0)
                nc.scalar.activation(
                    out=z_out[:tw, :], in_=xt[:tw, :], func=Identity,
                    bias=nmr[:tw], scale=r[:tw],
                )

            # ---------------- Phase A: LN1 + transpose + QKV ----------------
            with (
                tc.tile_pool(name="wA", bufs=1) as wA,
                tc.tile_pool(name="zTp", bufs=1) as zTp,
                tc.tile_pool(name="pA", bufs=3) as pA,
                tc.tile_pool(name="outA", bufs=4) as outA,
                tc.tile_pool(name="psA", bufs=3, space="PSUM") as psA,
                tc.tile_pool(name="tpA", bufs=2, space="PSUM") as tpA,
            ):
                wqk = wA.tile([128, 8, 16, 128], F16)
                nc.sync.dma_start(out=wqk, in_=wqk_d[:, :, :, :])
                wv = wA.tile([128, 8, E], F16)
                nc.sync.dma_start(out=wv, in_=wv_d[:, :, :])
                zT = zTp.tile([128, 8, ntok], F16)

                for t0, tw in _token_tiles(ntok, 128):
                    xt = pA.tile([128, E], F32, tag="xa")
                    nc.sync.dma_start(out=xt[:tw], in_=x_d[t0 : t0 + tw, :])
                    zt = pA.tile([128, E], F16, tag="za")
                    layer_norm(pA, xt, tw, zt)
                    for ec in range(8):
                        tp = tpA.tile([128, 128], F16)
                        nc.tensor.transpose(
                            tp[:, :tw], zt[:tw, ec * 128 : (ec + 1) * 128],
                            ident[:tw, :tw],
                        )
                        nc.vector.tensor_copy(zT[:, ec, t0 : t0 + tw], tp[:, :tw])

                # q,k: weights stationary, zT moving -> feature-major outputs
                for g0, gw in _token_tiles(ntok, 512):
                    for mc in range(16):
                        ps = psA.tile([128, 512], F32, tag="mmA")
                        for kc in range(8):
                            nc.tensor.matmul(
                                ps[:, :gw], wqk[:, kc, mc, :], zT[:, kc, g0 : g0 + gw],
                                start=(kc == 0), stop=(kc == 7),
                            )
                        ob = outA.tile([128, 512], F16, tag="qk_out")
                        nc.scalar.copy(ob[:, :gw], ps[:, :gw])
                        dst = qT_s if mc < 8 else kT_s
                        nc.sync.dma_start(
                            out=dst[:, mc % 8, g0 : g0 + gw], in_=ob[:, :gw]
                        )
                # v: zT chunks stationary, wv moving -> token-major output
                for t0, tw in _token_tiles(ntok, 128):
                    vt = outA.tile([128, H, 65], F16, tag="v_out")
                    nc.vector.memset(vt[:tw, :, 64:65], 1.0)
                    for nb in range(2):
                        ps = psA.tile([128, 512], F32, tag="mmA")
                        for kc in range(8):
                            nc.tensor.matmul(
                                ps[:tw], zT[:, kc, t0 : t0 + tw],
                                wv[:, kc, nb * 512 : (nb + 1) * 512],
                                start=(kc == 0), stop=(kc == 7),
                            )
                        nc.scalar.copy(
                            vt[:tw, nb * 8 : (nb + 1) * 8, 0:64],
                            ps[:tw].rearrange("p (h d) -> p h d", h=8),
                        )
                    nc.sync.dma_start(
                        out=v_s[t0 : t0 + tw, :],
                        in_=vt[:tw].rearrange("p h d -> p (h d)"),
                    )

            # ---------------- Phase B: attention ----------------
            SCH = [(0, 128), (128, 128), (256, 64)]  # (s0, sw) chunks of T=320
            with (
                tc.tile_pool(name="bqk", bufs=2) as bqk,
                tc.tile_pool(name="bv", bufs=2) as bv,
                tc.tile_pool(name="batn", bufs=2) as batn,
                tc.tile_pool(name="bwei", bufs=6) as bwei,
                tc.tile_pool(name="brs", bufs=4) as brs,
                tc.tile_pool(name="psS", bufs=2, space="PSUM") as psS,
                tc.tile_pool(name="psV", bufs=2, space="PSUM") as psV,
            ):
                for b in range(b_loc):
                    o = b * T
                    qTb = bqk.tile([128, 8, T], F16, tag="qTb")
                    nc.sync.dma_start(out=qTb, in_=qT_s[:, :, o : o + T])
                    kTb = bqk.tile([128, 8, T], F16, tag="kTb")
                    nc.sync.dma_start(out=kTb, in_=kT_s[:, :, o : o + T])
                    vb = bv.tile([128, 3, H * 65], F16, tag="vb")
                    for j, (s0, sw) in enumerate(SCH):
                        nc.sync.dma_start(
                            out=vb[:sw, j, :], in_=v_s[o + s0 : o + s0 + sw, :]
                        )
                    aTb = batn.tile([128, 8, T], F16, tag="aTb")
                    sums_all = brs.tile([128, 4, T], F32, tag="sums_all")
                    nc.vector.memset(sums_all, 1.0)
                    rinv_all = brs.tile([128, 4, T], F32, tag="rinv_all")
                    for h in range(H):
                        po = 64 * (h % 2)
                        ec = h // 2
                        qh = qTb[po : po + 64, ec, :]
                        kh = kTb[po : po + 64, ec, :]
                        weis = []
                        for j, (s0, sw) in enumerate(SCH):
                            span = T - s0
                            ps = psS.tile([128, 512], F32, tag="sco")
                            nc.tensor.matmul(
                                ps[:sw, :span], kh[:, s0 : s0 + sw], qh[:, s0:T],
                                start=True, stop=True,
                            )
                            nc.vector.tensor_add(
                                ps[:sw, :sw], ps[:sw, :sw], maskT[:sw, :sw]
                            )
                            wj = bwei.tile([128, T], F16, tag="wei")
                            nc.scalar.activation(
                                out=wj[:sw, :span], in_=ps[:sw, :span], func=Exp,
                                scale=float(E) ** -0.5,
                            )
                            weis.append((wj, s0, sw, span))
                        # fused attn+sums: lhsT = [v_h | ones] -> rows 0..63 attn,
                        # row 64 = column sums of wei
                        pav = psV.tile([128, 512], F32, tag="av")
                        for j, (wj, s0, sw, span) in enumerate(weis):
                            vh = vb[:sw, j, h * 65 : (h + 1) * 65]
                            nc.tensor.matmul(
                                pav[:65, s0:T], vh, wj[:sw, :span],
                                start=(j == 0), stop=(j == 2),
                                skip_group_check=True,
                            )
                        nc.vector.tensor_copy(aTb[po : po + 64, ec, :], pav[:64, :T])
                        nc.scalar.copy(
                            sums_all[32 * (h % 4) : 32 * (h % 4) + 1, h // 4, :],
                            pav[64:65, :T],
                        )
                    nc.vector.reciprocal(rinv_all, sums_all)
                    for ec in range(8):
                        rsb = brs.tile([128, T], F32, tag="rsb")
                        for half, h in ((0, 2 * ec), (64, 2 * ec + 1)):
                            rrow = rinv_all[
                                32 * (h % 4) : 32 * (h % 4) + 1, h // 4, :
                            ]
                            rrow_rep = bass.AP(
                                tensor=rrow.tensor, offset=rrow.offset,
                                ap=[list(rrow.ap[0]), [0, 64], [1, T]],
                            )
                            nc.gpsimd.dma_start(
                                out=rsb[half : half + 64, :], in_=rrow_rep
                            )
                        nc.vector.tensor_mul(aTb[:, ec, :], aTb[:, ec, :], rsb)
                    nc.sync.dma_start(out=aT_s[:, :, o : o + T], in_=aTb)

            # ---------------- Phase C: proj + residual + LN2 ----------------
            with (
                tc.tile_pool(name="wC", bufs=1) as wC,
                tc.tile_pool(name="cin", bufs=2) as cin,
                tc.tile_pool(name="cwk", bufs=3) as cwk,
                tc.tile_pool(name="czt", bufs=2) as czt,
                tc.tile_pool(name="psC", bufs=3, space="PSUM") as psC,
                tc.tile_pool(name="tpC", bufs=2, space="PSUM") as tpC,
            ):
                wo = wC.tile([128, 8, E], F16)
                nc.sync.dma_start(out=wo, in_=wo_d[:, :, :])
                for c0, cw in _token_tiles(ntok, 256):
                    aT = cin.tile([128, 8, 256], F16, tag="aT_in")
                    nc.sync.dma_start(out=aT[:, :, :cw], in_=aT_s[:, :, c0 : c0 + cw])
                    z2Tt = czt.tile([128, 8, 256], F16, tag="z2T_t")
                    for so, swd in _token_tiles(cw, 128):
                        xt = cwk.tile([128, E], F32, tag="xc")
                        nc.sync.dma_start(
                            out=xt[:swd], in_=x_d[c0 + so : c0 + so + swd, :]
                        )
                        x2t = cwk.tile([128, E], F32, tag="x2c")
                        for nb in range(2):
                            ps = psC.tile([128, 512], F32, tag="mmC")
                            for kc in range(8):
                                nc.tensor.matmul(
                                    ps[:swd], aT[:, kc, so : so + swd],
                                    wo[:, kc, nb * 512 : (nb + 1) * 512],
                                    start=(kc == 0), stop=(kc == 7),
                                )
                            nc.vector.tensor_add(
                                x2t[:swd, nb * 512 : (nb + 1) * 512], ps[:swd],
                                xt[:swd, nb * 512 : (nb + 1) * 512],
                            )
                        nc.vector.tensor_add(x2t[:swd], x2t[:swd], bo_b[:swd])
                        nc.sync.dma_start(
                            out=x2_s[c0 + so : c0 + so + swd, :], in_=x2t[:swd]
                        )
                        z2t = cwk.tile([128, E], F16, tag="z2c")
                        layer_norm(cwk, x2t, swd, z2t)
                        for ecc in range(8):
                            tp = tpC.tile([128, 128], F16)
                            nc.tensor.transpose(
                                tp[:, :swd], z2t[:swd, ecc * 128 : (ecc + 1) * 128],
                                ident[:swd, :swd],
                            )
                            nc.vector.tensor_copy(
                                z2Tt[:, ecc, so : so + swd], tp[:, :swd]
                            )
                    nc.sync.dma_start(
                        out=z2T_s[:, :, c0 : c0 + cw], in_=z2Tt[:, :, :cw]
                    )

            # ---------------- Phase D/E: FFN + residual ----------------
            with (
                tc.tile_pool(name="wDE", bufs=1) as wDE,
                tc.tile_pool(name="dh", bufs=2) as dh,
                tc.tile_pool(name="da1", bufs=1) as da1,
                tc.tile_pool(name="dy", bufs=2) as dy,
                tc.tile_pool(name="dout", bufs=2) as dout,
                tc.tile_pool(name="psD", bufs=3, space="PSUM") as psD,
                tc.tile_pool(name="tpD", bufs=2, space="PSUM") as tpD,
            ):
                w1 = wDE.tile([128, 8, 32, 128], F16)
                nc.sync.dma_start(out=w1, in_=w1_d[:, :, :, :])
                w2 = wDE.tile([128, 32, 8, 128], F16)
                nc.sync.dma_start(out=w2, in_=w2_d[:, :, :, :])
                for g0, gw in _token_tiles(ntok, 512):
                    hT = dh.tile([128, 8, 512], F16, tag="hT")
                    nc.sync.dma_start(
                        out=hT[:, :, :gw], in_=z2T_s[:, :, g0 : g0 + gw]
                    )
                    a1T = da1.tile([128, 32, 512], F16, tag="a1T")
                    for mc in range(32):
                        ps = psD.tile([128, 512], F32, tag="mmD")
                        for kc in range(8):
                            nc.tensor.matmul(
                                ps[:, :gw], w1[:, kc, mc, :], hT[:, kc, :gw],
                                start=(kc == 0), stop=(kc == 7),
                            )
                        nc.scalar.activation(
                            out=a1T[:, mc, :gw], in_=ps[:, :gw], func=Relu,
                            bias=b1t[:, mc : mc + 1], scale=1.0,
                        )
                    yT = dy.tile([128, 8, 512], F16, tag="yT")
                    for mc2 in range(8):
                        ps = psD.tile([128, 512], F32, tag="mmD")
                        for kc2 in range(32):
                            nc.tensor.matmul(
                                ps[:, :gw], w2[:, kc2, mc2, :], a1T[:, kc2, :gw],
                                start=(kc2 == 0), stop=(kc2 == 31),
                            )
                        nc.scalar.activation(
                            out=yT[:, mc2, :gw], in_=ps[:, :gw], func=Identity,
                            bias=b2t[:, mc2 : mc2 + 1], scale=1.0,
                        )
                    for so, swd in _token_tiles(gw, 128):
                        x2t = dout.tile([128, E], F32, tag="x2d")
                        nc.sync.dma_start(
                            out=x2t[:swd], in_=x2_s[g0 + so : g0 + so + swd, :]
                        )
                        pst = tpD.tile([128, 1024], F16)
                        for mc2 in range(8):
                            nc.tensor.transpose(
                                pst[:swd, mc2 * 128 : (mc2 + 1) * 128],
                                yT[:, mc2, so : so + swd],
                                ident[:, :],
                            )
                        nc.vector.tensor_add(x2t[:swd], pst[:swd], x2t[:swd])
                        nc.sync.dma_start(
                            out=out_d[g0 + so : g0 + so + swd, :], in_=x2t[:swd]
                        )
    return nc


def prep_weights(inputs):
    """Host-side weight preparation (fp16 casts, LN folding, layouts)."""
    f32 = np.float32
    Wq = np.asarray(inputs["Wq"], f32)
    Wk = np.asarray(inputs["Wk"], f32)
    Wv = np.asarray(inputs["Wv"], f32)
    Wo = np.asarray(inputs["Wo"], f32)
    bo = np.asarray(inputs["bo"], f32)
    W1 = np.asarray(inputs["W1"], f32)
    b1 = np.asarray(inputs["b1"], f32)
    W2 = np.asarray(inputs["W2"], f32)
    b2 = np.asarray(inputs["b2"], f32)
    ln1_w = np.asarray(inputs["ln1_w"], f32)
    ln1_b = np.asarray(inputs["ln1_b"], f32)
    ln2_w = np.asarray(inputs["ln2_w"], f32)
    ln2_b = np.asarray(inputs["ln2_b"], f32)

    assert np.all(ln1_b == 0.0) and np.all(ln2_b == 0.0), (
        "nonzero LN bias not supported by this kernel build"
    )

    # h = z*w + b with b==0  ->  fold w into the next matmul's weights (exact)
    Wq_all = np.transpose(Wq, (1, 0, 2)).reshape(E, E) * ln1_w[:, None]
    Wk_all = np.transpose(Wk, (1, 0, 2)).reshape(E, E) * ln1_w[:, None]
    Wv_all = np.transpose(Wv, (1, 0, 2)).reshape(E, E) * ln1_w[:, None]
    W1f = W1 * ln2_w[:, None]

    wqk = np.concatenate([Wq_all, Wk_all], axis=1)  # [E, 2048]
    wqk = wqk.reshape(8, 128, 16, 128).transpose(1, 0, 2, 3)
    wv = Wv_all.reshape(8, 128, E).transpose(1, 0, 2)
    wo = Wo.reshape(8, 128, E).transpose(1, 0, 2)
    w1 = W1f.reshape(8, 128, 32, 128).transpose(1, 0, 2, 3)
    w2 = W2.reshape(32, 128, 8, 128).transpose(1, 0, 2, 3)

    r = np.arange(128)
    maskT = np.where(r[:, None] <= r[None, :], 0.0, -1e9).astype(f32)

    return {
        "wqk": np.ascontiguousarray(wqk, np.float16),
        "wv": np.ascontiguousarray(wv, np.float16),
        "wo": np.ascontiguousarray(wo, np.float16),
        "w1": np.ascontiguousarray(w1, np.float16),
        "w2": np.ascontiguousarray(w2, np.float16),
        "bo_bcast": np.ascontiguousarray(np.tile(bo, (128, 1)), f32),
        "b1t": np.ascontiguousarray(b1.reshape(32, 128).T, f32),
        "b2t": np.ascontiguousarray(b2.reshape(8, 128).T, f32),
        "maskT": maskT,
        "ident16": np.eye(128, dtype=np.float16),
    }


_prog_cache = {}


def _get_program(b_loc):
    if b_loc not in _prog_cache:
        _prog_cache[b_loc] = build_program(b_loc)
    return _prog_cache[b_loc]


def kernel(**inputs) -> np.ndarray:
    from concourse.bass_utils import run_bass_kernel_spmd

    x = np.asarray(inputs["x"], np.float32)  # [B, T, E]
    wmap = prep_weights(inputs)
    nc = _get_program(B_LOC)
    in_maps = []
    for c in range(NCORES):
        m = dict(wmap)
        m["x"] = np.ascontiguousarray(
            x[c * B_LOC : (c + 1) * B_LOC].reshape(B_LOC * T, E)
        )
        in_maps.append(m)
    res = run_bass_kernel_spmd(nc, in_maps, core_ids=list(range(NCORES)))
    out = np.stack([res.results[c]["out"] for c in range(NCORES)])
    return out.reshape(B, T, E).astype(np.float32)


# revision 10
# speedup vs baseline: 1.0932x; 1.0932x over previous
"""Trainium2 Bass kernel for a dense transformer block (B=64,T=320,E=1024,H=16).

Strategy: pure data-parallel over batch across 8 NeuronCores (8 batches/core,
no collectives).  Per core, one Bass program runs the whole block in phases:

  A: LN1 (token-major) -> transpose -> zT (feature-major, SBUF-resident)
     -> QKV matmuls (q,k feature-major to HBM; v token-major to HBM)
  B: per (batch, head): scoresT = k^T.T @ q^T  (s on partitions, t on free),
     causal mask on diagonal blocks, exp (no max-sub needed: LN bounds scores),
     column sums via ones-matmul on PE, 1/sum broadcast via 0-stride DMA,
     attn^T = v.T @ weiT accumulated over s-chunks, normalized on copy-out.
  C: proj token-major (attnT chunks stationary, Wo moving) + residual + LN2
     -> transpose -> z2T (feature-major) to HBM
  D/E: fc1 (W1 stationary, z2T moving) -> relu+bias -> a1T (SBUF) ->
     fc2 (W2 stationary, a1T moving) -> +b2 -> transpose -> +x2 -> out

All matmul operands are fp16 (PE runs fp16 at full 1 cyc/row with FWL);
accumulation is fp32 in PSUM; the residual stream stays fp32 end-to-end.
LayerNorm scale/bias are folded into the adjacent weights host-side (exact).
"""

import sys

for _p in ("/opt/trn_rl_repo", "/opt/pypackages"):
    if _p not in sys.path:
        sys.path.append(_p)

import numpy as np

import concourse.bass as bass
import concourse.mybir as mybir
import concourse.tile as tile
import bass_rust

B, T, E, H = 64, 320, 1024, 16
HS = E // H  # 64
FF = 4 * E  # 4096
NCORES = 8
B_LOC = B // NCORES  # 8
EPS = 1e-5
F16 = mybir.dt.float16
F32 = mybir.dt.float32

MAX_DRAIN_WAITS = 1  # this walrus allows 1 sync wait per TPB_CTRL instruction


class SplitDrainTileContext(tile.TileContext):
    """TileContext whose exit drain spreads its sem waits over NOP carriers
    (the toolchain's TPB_CTRL lowering rejects >1 sync wait on one inst)."""

    WAIT_LIMIT = 1

    def _add_instruction(self, inst):
        si = getattr(inst, "sync_info", None)
        lim = self.WAIT_LIMIT
        if si is not None and si.on_wait and len(si.on_wait) > lim:
            waits = list(si.on_wait)
            extra, keep = waits[:-lim], waits[-lim:]
            inst.sync_info = mybir.SyncInfo(
                on_wait=keep, on_update=list(si.on_update or [])
            )
            for i in range(0, len(extra), lim):
                nop = mybir.InstNoOp(
                    name=self.nc.get_next_instruction_name(), ins=[], outs=[]
                )
                nop.engine = inst.engine
                nop.sync_info = mybir.SyncInfo(
                    on_wait=extra[i : i + lim], on_update=[]
                )
                super()._add_instruction(nop)
        super()._add_instruction(inst)

    def _drain_and_barrier(self, tick_clock, wait_clock):
        nc = self.nc
        carriers = [nc.sync.nop(nofuse=True) for _ in range(64)]
        drain_inst = nc.sync.drain()
        wait_clock.add_sem_waits(
            drain_inst.ins, bass_rust.ScopedClock({None: tick_clock.global_clock})
        )
        si = drain_inst.ins.sync_info
        waits = list(si.on_wait or [])
        drain_inst.ins.sync_info = mybir.SyncInfo(
            on_wait=[], on_update=list(si.on_update or [])
        )
        ci = 0
        for i in range(0, len(waits), MAX_DRAIN_WAITS):
            chunk = waits[i : i + MAX_DRAIN_WAITS]
            carriers[ci].ins.sync_info = mybir.SyncInfo(on_wait=chunk, on_update=[])
            ci += 1
        nc.all_engine_barrier()
        assert self.sems is not None
        popped = nc._tile_sem_poison_stack.pop()
        assert popped is self._sem_poison
        nc.clear_and_free_semaphores(list(self.sems.allocated().values()))
        nc.all_engine_barrier()


def _token_tiles(n, w):
    return [(o, min(w, n - o)) for o in range(0, n, w)]


def build_program(b_loc=B_LOC):
    """Build the per-core Bass program for b_loc batches (b_loc*T tokens)."""
    ntok = b_loc * T
    nc = bass.Bass()

    x_d = nc.dram_tensor("x", [ntok, E], F32, kind="ExternalInput")
    wqk_d = nc.dram_tensor("wqk", [128, 8, 16, 128], F16, kind="ExternalInput")
    wv_d = nc.dram_tensor("wv", [128, 8, E], F16, kind="ExternalInput")
    wo_d = nc.dram_tensor("wo", [128, 8, E], F16, kind="ExternalInput")
    w1_d = nc.dram_tensor("w1", [128, 8, 32, 128], F16, kind="ExternalInput")
    w2_d = nc.dram_tensor("w2", [128, 32, 8, 128], F16, kind="ExternalInput")
    bo_b_d = nc.dram_tensor("bo_bcast", [128, E], F32, kind="ExternalInput")
    b1t_d = nc.dram_tensor("b1t", [128, 32], F32, kind="ExternalInput")
    b2t_d = nc.dram_tensor("b2t", [128, 8], F32, kind="ExternalInput")
    maskt_d = nc.dram_tensor("maskT", [128, 128], F32, kind="ExternalInput")
    ident_d = nc.dram_tensor("ident16", [128, 128], F16, kind="ExternalInput")
    out_d = nc.dram_tensor("out", [ntok, E], F32, kind="ExternalOutput")

    Identity = mybir.ActivationFunctionType.Identity
    Exp = mybir.ActivationFunctionType.Exp
    Relu = mybir.ActivationFunctionType.Relu
    Sqrt = mybir.ActivationFunctionType.Sqrt

    with SplitDrainTileContext(nc) as tc:
        ctx_consts = tc.tile_pool(name="consts", bufs=1)
        ctx_dram = tc.tile_pool(name="dram", bufs=1, space="DRAM")
        with ctx_consts as consts, ctx_dram as dram:
            # --- constants ---
            bo_b = consts.tile([128, E], F32)
            nc.sync.dma_start(out=bo_b, in_=bo_b_d[:, :])
            csml = consts.tile([128, 41 + 32], F32)
            b1t = csml[:, 0:32]
            nc.sync.dma_start(out=b1t, in_=b1t_d[:, :])
            b2t = csml[:, 32:40]
            nc.sync.dma_start(out=b2t, in_=b2t_d[:, :])
            eps_t = csml[:, 40:41]
            nc.vector.memset(eps_t, EPS)
            ones16 = csml[:, 41:42].bitcast(F16)[:, 0:1]
            nc.vector.memset(ones16, 1.0)
            maskT = consts.tile([128, 128], F32)
            nc.sync.dma_start(out=maskT, in_=maskt_d[:, :])
            ident = consts.tile([128, 128], F16)
            nc.sync.dma_start(out=ident, in_=ident_d[:, :])

            # --- DRAM scratch (Tile-tracked) ---
            qT_s = dram.tile([128, 8, ntok], F16)
            kT_s = dram.tile([128, 8, ntok], F16)
            v_s = dram.tile([ntok, H * 65], F16)
            aT_s = dram.tile([128, 8, ntok], F16)
            x2_s = dram.tile([ntok, E], F32)
            z2T_s = dram.tile([128, 8, ntok], F16)

            def layer_norm(pool, xt, tw, z_out):
                """token-major LN of xt[:tw, :E] -> z_out[:tw, :E] (fp16)."""
                stats = pool.tile([128, 2, 6], F32, tag="ln_stats")
                nc.vector.bn_stats(out=stats[:tw, 0, :], in_=xt[:tw, 0:512])
                nc.vector.bn_stats(out=stats[:tw, 1, :], in_=xt[:tw, 512:1024])
                mv = pool.tile([128, 2], F32, tag="ln_mv")
                nc.vector.bn_aggr(out=mv[:tw], in_=stats[:tw])
                r = pool.tile([128, 1], F32, tag="ln_r")
                nc.scalar.activation(
                    out=r[:tw], in_=mv[:tw, 1:2], func=Sqrt, bias=eps_t[:tw], scale=1.0
                )
                nc.vector.reciprocal(out=r[:tw], in_=r[:tw])
                nmr = pool.tile([128, 1], F32, tag="ln_nmr")
                nc.vector.tensor_mul(nmr[:tw], mv[:tw, 0:1], r[:tw])
                nc.vector.tensor_scalar_mul(nmr[:tw], nmr[:tw], -1.0)
                nc.scalar.activation(
                    out=z_out[:tw, :], in_=xt[:tw, :], func=Identity,
                    bias=nmr[:tw], scale=r[:tw],
                )

            # ---------------- Phase A: LN1 + transpose + QKV ----------------
            with (
                tc.tile_pool(name="wA", bufs=1) as wA,
                tc.tile_pool(name="zTp", bufs=1) as zTp,
                tc.tile_pool(name="pA", bufs=3) as pA,
                tc.tile_pool(name="outA", bufs=4) as outA,
                tc.tile_pool(name="psA", bufs=3, space="PSUM") as psA,
                tc.tile_pool(name="tpA", bufs=2, space="PSUM") as tpA,
            ):
                wqk_k = []
                wv_k = []
                for kc in range(8):
                    wq1 = wA.tile([128, 16, 128], F16, name=f"wqk{kc}")
                    nc.sync.dma_start(out=wq1, in_=wqk_d[:, kc, :, :])
                    wqk_k.append(wq1)
                    wv1 = wA.tile([128, E], F16, name=f"wv{kc}")
                    nc.sync.dma_start(out=wv1, in_=wv_d[:, kc, :])
                    wv_k.append(wv1)
                zT = zTp.tile([128, 8, ntok], F16)

                for t0, tw in _token_tiles(ntok, 128):
                    xt = pA.tile([128, E], F32, tag="xa")
                    nc.sync.dma_start(out=xt[:tw], in_=x_d[t0 : t0 + tw, :])
                    zt = pA.tile([128, E], F16, tag="za")
                    layer_norm(pA, xt, tw, zt)
                    for ec in range(8):
                        tp = tpA.tile([128, 128], F16)
                        nc.tensor.transpose(
                            tp[:, :tw], zt[:tw, ec * 128 : (ec + 1) * 128],
                            ident[:tw, :tw],
                        )
                        nc.vector.tensor_copy(zT[:, ec, t0 : t0 + tw], tp[:, :tw])

                # q,k: weights stationary, zT moving -> feature-major outputs
                for g0, gw in _token_tiles(ntok, 512):
                    for mc in range(16):
                        ps = psA.tile([128, 512], F32, tag="mmA")
                        for kc in range(8):
                            nc.tensor.matmul(
                                ps[:, :gw], wqk_k[kc][:, mc, :], zT[:, kc, g0 : g0 + gw],
                                start=(kc == 0), stop=(kc == 7),
                            )
                        ob = outA.tile([128, 512], F16, tag="qk_out")
                        nc.scalar.copy(ob[:, :gw], ps[:, :gw])
                        dst = qT_s if mc < 8 else kT_s
                        nc.sync.dma_start(
                            out=dst[:, mc % 8, g0 : g0 + gw], in_=ob[:, :gw]
                        )
                # v: zT chunks stationary, wv moving -> token-major output
                for t0, tw in _token_tiles(ntok, 128):
                    vt = outA.tile([128, H, 65], F16, tag="v_out")
                    nc.vector.memset(vt[:tw, :, 64:65], 1.0)
                    for nb in range(2):
                        ps = psA.tile([128, 512], F32, tag="mmA")
                        for kc in range(8):
                            nc.tensor.matmul(
                                ps[:tw], zT[:, kc, t0 : t0 + tw],
                                wv_k[kc][:, nb * 512 : (nb + 1) * 512],
                                start=(kc == 0), stop=(kc == 7),
                            )
                        nc.scalar.copy(
                            vt[:tw, nb * 8 : (nb + 1) * 8, 0:64],
                            ps[:tw].rearrange("p (h d) -> p h d", h=8),
                        )
                    nc.sync.dma_start(
                        out=v_s[t0 : t0 + tw, :],
                        in_=vt[:tw].rearrange("p h d -> p (h d)"),
                    )

            # ---------------- Phase B: attention ----------------
            SCH = [(0, 128), (128, 128), (256, 64)]  # (s0, sw) chunks of T=320
            with (
                tc.tile_pool(name="bqk", bufs=2) as bqk,
                tc.tile_pool(name="bv", bufs=2) as bv,
                tc.tile_pool(name="batn", bufs=2) as batn,
                tc.tile_pool(name="bwei", bufs=9) as bwei,
                tc.tile_pool(name="brs", bufs=4) as brs,
                tc.tile_pool(name="psS", bufs=4, space="PSUM") as psS,
                tc.tile_pool(name="psV", bufs=3, space="PSUM") as psV,
            ):
                for b in range(b_loc):
                    o = b * T
                    qTb = bqk.tile([128, 8, T], F16, tag="qTb")
                    nc.sync.dma_start(out=qTb, in_=qT_s[:, :, o : o + T])
                    kTb = bqk.tile([128, 8, T], F16, tag="kTb")
                    nc.sync.dma_start(out=kTb, in_=kT_s[:, :, o : o + T])
                    vb = bv.tile([128, 3, H * 65], F16, tag="vb")
                    for j, (s0, sw) in enumerate(SCH):
                        nc.sync.dma_start(
                            out=vb[:sw, j, :], in_=v_s[o + s0 : o + s0 + sw, :]
                        )
                    aTb = batn.tile([128, 8, T], F16, tag="aTb")
                    sums_all = brs.tile([128, 4, T], F32, tag="sums_all")
                    nc.vector.memset(sums_all, 1.0)
                    rinv_all = brs.tile([128, 4, T], F32, tag="rinv_all")
                    for h in range(H):
                        po = 64 * (h % 2)
                        ec = h // 2
                        qh = qTb[po : po + 64, ec, :]
                        kh = kTb[po : po + 64, ec, :]
                        weis = []
                        for j, (s0, sw) in enumerate(SCH):
                            span = T - s0
                            ps = psS.tile([128, 512], F32, tag="sco")
                            nc.tensor.matmul(
                                ps[:sw, :span], kh[:, s0 : s0 + sw], qh[:, s0:T],
                                start=True, stop=True,
                            )
                            nc.vector.tensor_add(
                                ps[:sw, :sw], ps[:sw, :sw], maskT[:sw, :sw]
                            )
                            wj = bwei.tile([128, T], F16, tag="wei")
                            nc.scalar.activation(
                                out=wj[:sw, :span], in_=ps[:sw, :span], func=Exp,
                                scale=float(E) ** -0.5,
                            )
                            weis.append((wj, s0, sw, span))
                        # fused attn+sums: lhsT = [v_h | ones] -> rows 0..63 attn,
                        # row 64 = column sums of wei
                        pav = psV.tile([128, 512], F32, tag="av")
                        for j, (wj, s0, sw, span) in enumerate(weis):
                            vh = vb[:sw, j, h * 65 : (h + 1) * 65]
                            nc.tensor.matmul(
                                pav[:65, s0:T], vh, wj[:sw, :span],
                                start=(j == 0), stop=(j == 2),
                                skip_group_check=True,
                            )
                        nc.vector.tensor_copy(aTb[po : po + 64, ec, :], pav[:64, :T])
                        nc.scalar.copy(
                            sums_all[32 * (h % 4) : 32 * (h % 4) + 1, h // 4, :],
                            pav[64:65, :T],
                        )
                    nc.vector.reciprocal(rinv_all, sums_all)
                    for ec in range(8):
                        rsb = brs.tile([128, T], F32, tag="rsb")
                        for half, h in ((0, 2 * ec), (64, 2 * ec + 1)):
                            rrow = rinv_all[
                                32 * (h % 4) : 32 * (h % 4) + 1, h // 4, :
                            ]
                            rrow_rep = bass.AP(
                                tensor=rrow.tensor, offset=rrow.offset,
                                ap=[list(rrow.ap[0]), [0, 64], [1, T]],
                            )
                            nc.gpsimd.dma_start(
                                out=rsb[half : half + 64, :], in_=rrow_rep
                            )
                        nc.vector.tensor_mul(aTb[:, ec, :], aTb[:, ec, :], rsb)
                    nc.sync.dma_start(out=aT_s[:, :, o : o + T], in_=aTb)

            # ---------------- Phase C: proj + residual + LN2 ----------------
            with (
                tc.tile_pool(name="wC", bufs=1) as wC,
                tc.tile_pool(name="cin", bufs=2) as cin,
                tc.tile_pool(name="cwk", bufs=3) as cwk,
                tc.tile_pool(name="czt", bufs=2) as czt,
                tc.tile_pool(name="psC", bufs=3, space="PSUM") as psC,
                tc.tile_pool(name="tpC", bufs=2, space="PSUM") as tpC,
            ):
                wo_k = []
                for kc in range(8):
                    wo1 = wC.tile([128, E], F16, name=f"wo{kc}")
                    nc.sync.dma_start(out=wo1, in_=wo_d[:, kc, :])
                    wo_k.append(wo1)
                for c0, cw in _token_tiles(ntok, 256):
                    aT = cin.tile([128, 8, 256], F16, tag="aT_in")
                    nc.sync.dma_start(out=aT[:, :, :cw], in_=aT_s[:, :, c0 : c0 + cw])
                    z2Tt = czt.tile([128, 8, 256], F16, tag="z2T_t")
                    for so, swd in _token_tiles(cw, 128):
                        xt = cwk.tile([128, E], F32, tag="xc")
                        nc.sync.dma_start(
                            out=xt[:swd], in_=x_d[c0 + so : c0 + so + swd, :]
                        )
                        x2t = cwk.tile([128, E], F32, tag="x2c")
                        for nb in range(2):
                            ps = psC.tile([128, 512], F32, tag="mmC")
                            for kc in range(8):
                                nc.tensor.matmul(
                                    ps[:swd], aT[:, kc, so : so + swd],
                                    wo_k[kc][:, nb * 512 : (nb + 1) * 512],
                                    start=(kc == 0), stop=(kc == 7),
                                )
                            nc.vector.tensor_add(
                                x2t[:swd, nb * 512 : (nb + 1) * 512], ps[:swd],
                                xt[:swd, nb * 512 : (nb + 1) * 512],
                            )
                        nc.vector.tensor_add(x2t[:swd], x2t[:swd], bo_b[:swd])
                        nc.sync.dma_start(
                            out=x2_s[c0 + so : c0 + so + swd, :], in_=x2t[:swd]
                        )
                        z2t = cwk.tile([128, E], F16, tag="z2c")
                        layer_norm(cwk, x2t, swd, z2t)
                        for ecc in range(8):
                            tp = tpC.tile([128, 128], F16)
                            nc.tensor.transpose(
                                tp[:, :swd], z2t[:swd, ecc * 128 : (ecc + 1) * 128],
                                ident[:swd, :swd],
                            )
                            nc.vector.tensor_copy(
                                z2Tt[:, ecc, so : so + swd], tp[:, :swd]
                            )
                    nc.sync.dma_start(
                        out=z2T_s[:, :, c0 : c0 + cw], in_=z2Tt[:, :, :cw]
                    )

            # ---------------- Phase D/E: FFN + residual ----------------
            with (
                tc.tile_pool(name="wDE", bufs=1) as wDE,
                tc.tile_pool(name="dh", bufs=2) as dh,
                tc.tile_pool(name="da1", bufs=1) as da1,
                tc.tile_pool(name="dy", bufs=2) as dy,
                tc.tile_pool(name="dout", bufs=2) as dout,
                tc.tile_pool(name="psD", bufs=3, space="PSUM") as psD,
                tc.tile_pool(name="tpD", bufs=2, space="PSUM") as tpD,
            ):
                w1_k = []
                for kc in range(8):
                    w11 = wDE.tile([128, 32, 128], F16, name=f"w1_{kc}")
                    nc.sync.dma_start(out=w11, in_=w1_d[:, kc, :, :])
                    w1_k.append(w11)
                w2_k = []
                for kg in range(4):
                    w21 = wDE.tile([128, 8, 8, 128], F16, name=f"w2_{kg}")
                    nc.sync.dma_start(out=w21, in_=w2_d[:, kg * 8 : (kg + 1) * 8, :, :])
                    w2_k.append(w21)
                for g0, gw in _token_tiles(ntok, 512):
                    hT = dh.tile([128, 8, 512], F16, tag="hT")
                    nc.sync.dma_start(
                        out=hT[:, :, :gw], in_=z2T_s[:, :, g0 : g0 + gw]
                    )
                    a1T = da1.tile([128, 32, 512], F16, tag="a1T")
                    for mc in range(32):
                        ps = psD.tile([128, 512], F32, tag="mmD")
                        for kc in range(8):
                            nc.tensor.matmul(
                                ps[:, :gw], w1_k[kc][:, mc, :], hT[:, kc, :gw],
                                start=(kc == 0), stop=(kc == 7),
                            )
                        nc.scalar.activation(
                            out=a1T[:, mc, :gw], in_=ps[:, :gw], func=Relu,
                            bias=b1t[:, mc : mc + 1], scale=1.0,
                        )
                    yT = dy.tile([128, 8, 512], F16, tag="yT")
                    for mc2 in range(8):
                        ps = psD.tile([128, 512], F32, tag="mmD")
                        for kc2 in range(32):
                            nc.tensor.matmul(
                                ps[:, :gw], w2_k[kc2 // 8][:, kc2 % 8, mc2, :], a1T[:, kc2, :gw],
                                start=(kc2 == 0), stop=(kc2 == 31),
                            )
                        nc.scalar.activation(
                            out=yT[:, mc2, :gw], in_=ps[:, :gw], func=Identity,
                            bias=b2t[:, mc2 : mc2 + 1], scale=1.0,
                        )
                    for so, swd in _token_tiles(gw, 128):
                        x2t = dout.tile([128, E], F32, tag="x2d")
                        nc.sync.dma_start(
                            out=x2t[:swd], in_=x2_s[g0 + so : g0 + so + swd, :]
                        )
                        pst = tpD.tile([128, 1024], F16)
                        for mc2 in range(8):
                            nc.tensor.transpose(
                                pst[:swd, mc2 * 128 : (mc2 + 1) * 128],
                                yT[:, mc2, so : so + swd],
                                ident[:, :],
                            )
                        nc.vector.tensor_add(x2t[:swd], pst[:swd], x2t[:swd])
                        nc.sync.dma_start(
                            out=out_d[g0 + so : g0 + so + swd, :], in_=x2t[:swd]
                        )
    return nc


def prep_weights(inputs):
    """Host-side weight preparation (fp16 casts, LN folding, layouts)."""
    f32 = np.float32
    Wq = np.asarray(inputs["Wq"], f32)
    Wk = np.asarray(inputs["Wk"], f32)
    Wv = np.asarray(inputs["Wv"], f32)
    Wo = np.asarray(inputs["Wo"], f32)
    bo = np.asarray(inputs["bo"], f32)
    W1 = np.asarray(inputs["W1"], f32)
    b1 = np.asarray(inputs["b1"], f32)
    W2 = np.asarray(inputs["W2"], f32)
    b2 = np.asarray(inputs["b2"], f32)
    ln1_w = np.asarray(inputs["ln1_w"], f32)
    ln1_b = np.asarray(inputs["ln1_b"], f32)
    ln2_w = np.asarray(inputs["ln2_w"], f32)
    ln2_b = np.asarray(inputs["ln2_b"], f32)

    assert np.all(ln1_b == 0.0) and np.all(ln2_b == 0.0), (
        "nonzero LN bias not supported by this kernel build"
    )

    # h = z*w + b with b==0  ->  fold w into the next matmul's weights (exact)
    Wq_all = np.transpose(Wq, (1, 0, 2)).reshape(E, E) * ln1_w[:, None]
    Wk_all = np.transpose(Wk, (1, 0, 2)).reshape(E, E) * ln1_w[:, None]
    Wv_all = np.transpose(Wv, (1, 0, 2)).reshape(E, E) * ln1_w[:, None]
    W1f = W1 * ln2_w[:, None]

    wqk = np.concatenate([Wq_all, Wk_all], axis=1)  # [E, 2048]
    wqk = wqk.reshape(8, 128, 16, 128).transpose(1, 0, 2, 3)
    wv = Wv_all.reshape(8, 128, E).transpose(1, 0, 2)
    wo = Wo.reshape(8, 128, E).transpose(1, 0, 2)
    w1 = W1f.reshape(8, 128, 32, 128).transpose(1, 0, 2, 3)
    w2 = W2.reshape(32, 128, 8, 128).transpose(1, 0, 2, 3)

    r = np.arange(128)
    maskT = np.where(r[:, None] <= r[None, :], 0.0, -1e9).astype(f32)

    return {
        "wqk": np.ascontiguousarray(wqk, np.float16),
        "wv": np.ascontiguousarray(wv, np.float16),
        "wo": np.ascontiguousarray(wo, np.float16),
        "w1": np.ascontiguousarray(w1, np.float16),
        "w2": np.ascontiguousarray(w2, np.float16),
        "bo_bcast": np.ascontiguousarray(np.tile(bo, (128, 1)), f32),
        "b1t": np.ascontiguousarray(b1.reshape(32, 128).T, f32),
        "b2t": np.ascontiguousarray(b2.reshape(8, 128).T, f32),
        "maskT": maskT,
        "ident16": np.eye(128, dtype=np.float16),
    }


_prog_cache = {}


def _get_program(b_loc):
    if b_loc not in _prog_cache:
        _prog_cache[b_loc] = build_program(b_loc)
    return _prog_cache[b_loc]


def kernel(**inputs) -> np.ndarray:
    from concourse.bass_utils import run_bass_kernel_spmd

    x = np.asarray(inputs["x"], np.float32)  # [B, T, E]
    wmap = prep_weights(inputs)
    nc = _get_program(B_LOC)
    in_maps = []
    for c in range(NCORES):
        m = dict(wmap)
        m["x"] = np.ascontiguousarray(
            x[c * B_LOC : (c + 1) * B_LOC].reshape(B_LOC * T, E)
        )
        in_maps.append(m)
    res = run_bass_kernel_spmd(nc, in_maps, core_ids=list(range(NCORES)))
    out = np.stack([res.results[c]["out"] for c in range(NCORES)])
    return out.reshape(B, T, E).astype(np.float32)


# revision 11
# speedup vs baseline: 1.0957x; 1.0022x over previous
"""Trainium2 Bass kernel for a dense transformer block (B=64,T=320,E=1024,H=16).

Strategy: pure data-parallel over batch across 8 NeuronCores (8 batches/core,
no collectives).  Per core, one Bass program runs the whole block in phases:

  A: LN1 (token-major) -> transpose -> zT (feature-major, SBUF-resident)
     -> QKV matmuls (q,k feature-major to HBM; v token-major to HBM)
  B: per (batch, head): scoresT = k^T.T @ q^T  (s on partitions, t on free),
     causal mask on diagonal blocks, exp (no max-sub needed: LN bounds scores),
     column sums via ones-matmul on PE, 1/sum broadcast via 0-stride DMA,
     attn^T = v.T @ weiT accumulated over s-chunks, normalized on copy-out.
  C: proj token-major (attnT chunks stationary, Wo moving) + residual + LN2
     -> transpose -> z2T (feature-major) to HBM
  D/E: fc1 (W1 stationary, z2T moving) -> relu+bias -> a1T (SBUF) ->
     fc2 (W2 stationary, a1T moving) -> +b2 -> transpose -> +x2 -> out

All matmul operands are fp16 (PE runs fp16 at full 1 cyc/row with FWL);
accumulation is fp32 in PSUM; the residual stream stays fp32 end-to-end.
LayerNorm scale/bias are folded into the adjacent weights host-side (exact).
"""

import sys

for _p in ("/opt/trn_rl_repo", "/opt/pypackages"):
    if _p not in sys.path:
        sys.path.append(_p)

import numpy as np

import concourse.bass as bass
import concourse.mybir as mybir
import concourse.tile as tile
import bass_rust

B, T, E, H = 64, 320, 1024, 16
HS = E // H  # 64
FF = 4 * E  # 4096
NCORES = 8
B_LOC = B // NCORES  # 8
EPS = 1e-5
F16 = mybir.dt.float16
F32 = mybir.dt.float32

MAX_DRAIN_WAITS = 1  # this walrus allows 1 sync wait per TPB_CTRL instruction


class SplitDrainTileContext(tile.TileContext):
    """TileContext whose exit drain spreads its sem waits over NOP carriers
    (the toolchain's TPB_CTRL lowering rejects >1 sync wait on one inst)."""

    WAIT_LIMIT = 1

    def _add_instruction(self, inst):
        si = getattr(inst, "sync_info", None)
        lim = self.WAIT_LIMIT
        if si is not None and si.on_wait and len(si.on_wait) > lim:
            waits = list(si.on_wait)
            extra, keep = waits[:-lim], waits[-lim:]
            inst.sync_info = mybir.SyncInfo(
                on_wait=keep, on_update=list(si.on_update or [])
            )
            for i in range(0, len(extra), lim):
                nop = mybir.InstNoOp(
                    name=self.nc.get_next_instruction_name(), ins=[], outs=[]
                )
                nop.engine = inst.engine
                nop.sync_info = mybir.SyncInfo(
                    on_wait=extra[i : i + lim], on_update=[]
                )
                super()._add_instruction(nop)
        super()._add_instruction(inst)

    def _drain_and_barrier(self, tick_clock, wait_clock):
        nc = self.nc
        carriers = [nc.sync.nop(nofuse=True) for _ in range(64)]
        drain_inst = nc.sync.drain()
        wait_clock.add_sem_waits(
            drain_inst.ins, bass_rust.ScopedClock({None: tick_clock.global_clock})
        )
        si = drain_inst.ins.sync_info
        waits = list(si.on_wait or [])
        drain_inst.ins.sync_info = mybir.SyncInfo(
            on_wait=[], on_update=list(si.on_update or [])
        )
        ci = 0
        for i in range(0, len(waits), MAX_DRAIN_WAITS):
            chunk = waits[i : i + MAX_DRAIN_WAITS]
            carriers[ci].ins.sync_info = mybir.SyncInfo(on_wait=chunk, on_update=[])
            ci += 1
        nc.all_engine_barrier()
        assert self.sems is not None
        popped = nc._tile_sem_poison_stack.pop()
        assert popped is self._sem_poison
        nc.clear_and_free_semaphores(list(self.sems.allocated().values()))
        nc.all_engine_barrier()


def _token_tiles(n, w):
    return [(o, min(w, n - o)) for o in range(0, n, w)]


def build_program(b_loc=B_LOC):
    """Build the per-core Bass program for b_loc batches (b_loc*T tokens)."""
    ntok = b_loc * T
    nc = bass.Bass()

    x_d = nc.dram_tensor("x", [ntok, E], F32, kind="ExternalInput")
    wqk_d = nc.dram_tensor("wqk", [128, 8, 16, 128], F16, kind="ExternalInput")
    wv_d = nc.dram_tensor("wv", [128, 8, E], F16, kind="ExternalInput")
    wo_d = nc.dram_tensor("wo", [128, 8, E], F16, kind="ExternalInput")
    w1_d = nc.dram_tensor("w1", [128, 8, 32, 128], F16, kind="ExternalInput")
    w2_d = nc.dram_tensor("w2", [128, 32, 8, 128], F16, kind="ExternalInput")
    bo_b_d = nc.dram_tensor("bo_bcast", [128, E], F32, kind="ExternalInput")
    b1t_d = nc.dram_tensor("b1t", [128, 32], F32, kind="ExternalInput")
    b2t_d = nc.dram_tensor("b2t", [128, 8], F32, kind="ExternalInput")
    maskt_d = nc.dram_tensor("maskT", [128, 128], F32, kind="ExternalInput")
    ident_d = nc.dram_tensor("ident16", [128, 128], F16, kind="ExternalInput")
    out_d = nc.dram_tensor("out", [ntok, E], F32, kind="ExternalOutput")

    Identity = mybir.ActivationFunctionType.Identity
    Exp = mybir.ActivationFunctionType.Exp
    Relu = mybir.ActivationFunctionType.Relu
    Sqrt = mybir.ActivationFunctionType.Sqrt

    with SplitDrainTileContext(nc) as tc:
        ctx_consts = tc.tile_pool(name="consts", bufs=1)
        ctx_dram = tc.tile_pool(name="dram", bufs=1, space="DRAM")
        with ctx_consts as consts, ctx_dram as dram:
            # --- constants ---
            bo_b = consts.tile([128, E], F32)
            nc.sync.dma_start(out=bo_b, in_=bo_b_d[:, :])
            csml = consts.tile([128, 41 + 32], F32)
            b1t = csml[:, 0:32]
            nc.sync.dma_start(out=b1t, in_=b1t_d[:, :])
            b2t = csml[:, 32:40]
            nc.sync.dma_start(out=b2t, in_=b2t_d[:, :])
            eps_t = csml[:, 40:41]
            nc.vector.memset(eps_t, EPS)
            ones16 = csml[:, 41:42].bitcast(F16)[:, 0:1]
            nc.vector.memset(ones16, 1.0)
            maskT = consts.tile([128, 128], F32)
            nc.sync.dma_start(out=maskT, in_=maskt_d[:, :])
            ident = consts.tile([128, 128], F16)
            nc.sync.dma_start(out=ident, in_=ident_d[:, :])

            # --- DRAM scratch (Tile-tracked) ---
            qT_s = dram.tile([128, 8, ntok], F16)
            kT_s = dram.tile([128, 8, ntok], F16)
            v_s = dram.tile([ntok, H * 65], F16)
            aT_s = dram.tile([128, 8, ntok], F16)
            x2_s = dram.tile([ntok, E], F32)
            z2T_s = dram.tile([128, 8, ntok], F16)

            def layer_norm(pool, xt, tw, z_out):
                """token-major LN of xt[:tw, :E] -> z_out[:tw, :E] (fp16)."""
                stats = pool.tile([128, 2, 6], F32, tag="ln_stats")
                nc.vector.bn_stats(out=stats[:tw, 0, :], in_=xt[:tw, 0:512])
                nc.vector.bn_stats(out=stats[:tw, 1, :], in_=xt[:tw, 512:1024])
                mv = pool.tile([128, 2], F32, tag="ln_mv")
                nc.vector.bn_aggr(out=mv[:tw], in_=stats[:tw])
                r = pool.tile([128, 1], F32, tag="ln_r")
                nc.scalar.activation(
                    out=r[:tw], in_=mv[:tw, 1:2], func=Sqrt, bias=eps_t[:tw], scale=1.0
                )
                nc.vector.reciprocal(out=r[:tw], in_=r[:tw])
                nmr = pool.tile([128, 1], F32, tag="ln_nmr")
                nc.vector.tensor_mul(nmr[:tw], mv[:tw, 0:1], r[:tw])
                nc.vector.tensor_scalar_mul(nmr[:tw], nmr[:tw], -1.0)
                nc.scalar.activation(
                    out=z_out[:tw, :], in_=xt[:tw, :], func=Identity,
                    bias=nmr[:tw], scale=r[:tw],
                )

            # ---------------- Phase A: LN1 + transpose + QKV ----------------
            with (
                tc.tile_pool(name="wA", bufs=1) as wA,
                tc.tile_pool(name="zTp", bufs=1) as zTp,
                tc.tile_pool(name="pA", bufs=3) as pA,
                tc.tile_pool(name="outA", bufs=4) as outA,
                tc.tile_pool(name="psA", bufs=3, space="PSUM") as psA,
                tc.tile_pool(name="tpA", bufs=2, space="PSUM") as tpA,
            ):
                wqk_k = []
                wv_k = []
                for kc in range(8):
                    wq1 = wA.tile([128, 16, 128], F16, name=f"wqk{kc}")
                    nc.sync.dma_start(out=wq1, in_=wqk_d[:, kc, :, :])
                    wqk_k.append(wq1)
                    wv1 = wA.tile([128, E], F16, name=f"wv{kc}")
                    nc.sync.dma_start(out=wv1, in_=wv_d[:, kc, :])
                    wv_k.append(wv1)
                zT = zTp.tile([128, 8, ntok], F16)

                for t0, tw in _token_tiles(ntok, 128):
                    xt = pA.tile([128, E], F32, tag="xa")
                    nc.sync.dma_start(out=xt[:tw], in_=x_d[t0 : t0 + tw, :])
                    zt = pA.tile([128, E], F16, tag="za")
                    layer_norm(pA, xt, tw, zt)
                    for ec in range(8):
                        tp = tpA.tile([128, 128], F16)
                        nc.tensor.transpose(
                            tp[:, :tw], zt[:tw, ec * 128 : (ec + 1) * 128],
                            ident[:tw, :tw],
                        )
                        nc.vector.tensor_copy(zT[:, ec, t0 : t0 + tw], tp[:, :tw])

                # q,k: weights stationary, zT moving -> feature-major outputs
                for g0, gw in _token_tiles(ntok, 512):
                    for mc in range(16):
                        ps = psA.tile([128, 512], F32, tag="mmA")
                        for kc in range(8):
                            nc.tensor.matmul(
                                ps[:, :gw], wqk_k[kc][:, mc, :], zT[:, kc, g0 : g0 + gw],
                                start=(kc == 0), stop=(kc == 7),
                            )
                        ob = outA.tile([128, 512], F16, tag="qk_out")
                        nc.scalar.copy(ob[:, :gw], ps[:, :gw])
                        dst = qT_s if mc < 8 else kT_s
                        nc.sync.dma_start(
                            out=dst[:, mc % 8, g0 : g0 + gw], in_=ob[:, :gw]
                        )
                # v: zT chunks stationary, wv moving -> token-major output
                for t0, tw in _token_tiles(ntok, 128):
                    vt = outA.tile([128, H, 65], F16, tag="v_out")
                    nc.vector.memset(vt[:tw, :, 64:65], 1.0)
                    for nb in range(2):
                        ps = psA.tile([128, 512], F32, tag="mmA")
                        for kc in range(8):
                            nc.tensor.matmul(
                                ps[:tw], zT[:, kc, t0 : t0 + tw],
                                wv_k[kc][:, nb * 512 : (nb + 1) * 512],
                                start=(kc == 0), stop=(kc == 7),
                            )
                        nc.scalar.copy(
                            vt[:tw, nb * 8 : (nb + 1) * 8, 0:64],
                            ps[:tw].rearrange("p (h d) -> p h d", h=8),
                        )
                    nc.sync.dma_start(
                        out=v_s[t0 : t0 + tw, :],
                        in_=vt[:tw].rearrange("p h d -> p (h d)"),
                    )

            # ---------------- Phase B: attention ----------------
            SCH = [(0, 128), (128, 128), (256, 64)]  # (s0, sw) chunks of T=320
            with (
                tc.tile_pool(name="bqk", bufs=2) as bqk,
                tc.tile_pool(name="bv", bufs=2) as bv,
                tc.tile_pool(name="batn", bufs=2) as batn,
                tc.tile_pool(name="bwei", bufs=9) as bwei,
                tc.tile_pool(name="brs", bufs=4) as brs,
                tc.tile_pool(name="psS", bufs=5, space="PSUM") as psS,
                tc.tile_pool(name="psV", bufs=3, space="PSUM") as psV,
            ):
                for b in range(b_loc):
                    o = b * T
                    qTb = bqk.tile([128, 8, T], F16, tag="qTb")
                    nc.sync.dma_start(out=qTb, in_=qT_s[:, :, o : o + T])
                    kTb = bqk.tile([128, 8, T], F16, tag="kTb")
                    nc.sync.dma_start(out=kTb, in_=kT_s[:, :, o : o + T])
                    vb = bv.tile([128, 3, H * 65], F16, tag="vb")
                    for j, (s0, sw) in enumerate(SCH):
                        nc.sync.dma_start(
                            out=vb[:sw, j, :], in_=v_s[o + s0 : o + s0 + sw, :]
                        )
                    aTb = batn.tile([128, 8, T], F16, tag="aTb")
                    sums_all = brs.tile([128, 4, T], F32, tag="sums_all")
                    nc.vector.memset(sums_all, 1.0)
                    rinv_all = brs.tile([128, 4, T], F32, tag="rinv_all")
                    SKEW = 2
                    weis_by_head = {}
                    for hh in range(H + SKEW):
                        if hh < H:
                            h = hh
                            po = 64 * (h % 2)
                            ec = h // 2
                            qh = qTb[po : po + 64, ec, :]
                            kh = kTb[po : po + 64, ec, :]
                            weis = []
                            for j, (s0, sw) in enumerate(SCH):
                                span = T - s0
                                ps = psS.tile([128, 512], F32, tag="sco")
                                nc.tensor.matmul(
                                    ps[:sw, :span], kh[:, s0 : s0 + sw], qh[:, s0:T],
                                    start=True, stop=True,
                                )
                                nc.vector.tensor_add(
                                    ps[:sw, :sw], ps[:sw, :sw], maskT[:sw, :sw]
                                )
                                wj = bwei.tile([128, T], F16, tag="wei")
                                nc.scalar.activation(
                                    out=wj[:sw, :span], in_=ps[:sw, :span],
                                    func=Exp, scale=float(E) ** -0.5,
                                )
                                weis.append((wj, s0, sw, span))
                            weis_by_head[h] = weis
                        if hh >= SKEW:
                            h = hh - SKEW
                            po = 64 * (h % 2)
                            ec = h // 2
                            weis = weis_by_head.pop(h)
                            # fused attn+sums: lhsT = [v_h | ones] -> rows 0..63
                            # attn, row 64 = column sums of wei
                            pav = psV.tile([128, 512], F32, tag="av")
                            for j, (wj, s0, sw, span) in enumerate(weis):
                                vh = vb[:sw, j, h * 65 : (h + 1) * 65]
                                nc.tensor.matmul(
                                    pav[:65, s0:T], vh, wj[:sw, :span],
                                    start=(j == 0), stop=(j == 2),
                                    skip_group_check=True,
                                )
                            nc.vector.tensor_copy(
                                aTb[po : po + 64, ec, :], pav[:64, :T]
                            )
                            nc.scalar.copy(
                                sums_all[32 * (h % 4) : 32 * (h % 4) + 1, h // 4, :],
                                pav[64:65, :T],
                            )
                    nc.vector.reciprocal(rinv_all, sums_all)
                    for ec in range(8):
                        rsb = brs.tile([128, T], F32, tag="rsb")
                        for half, h in ((0, 2 * ec), (64, 2 * ec + 1)):
                            rrow = rinv_all[
                                32 * (h % 4) : 32 * (h % 4) + 1, h // 4, :
                            ]
                            rrow_rep = bass.AP(
                                tensor=rrow.tensor, offset=rrow.offset,
                                ap=[list(rrow.ap[0]), [0, 64], [1, T]],
                            )
                            nc.gpsimd.dma_start(
                                out=rsb[half : half + 64, :], in_=rrow_rep
                            )
                        nc.vector.tensor_mul(aTb[:, ec, :], aTb[:, ec, :], rsb)
                    nc.sync.dma_start(out=aT_s[:, :, o : o + T], in_=aTb)

            # ---------------- Phase C: proj + residual + LN2 ----------------
            with (
                tc.tile_pool(name="wC", bufs=1) as wC,
                tc.tile_pool(name="cin", bufs=2) as cin,
                tc.tile_pool(name="cwk", bufs=3) as cwk,
                tc.tile_pool(name="czt", bufs=2) as czt,
                tc.tile_pool(name="psC", bufs=3, space="PSUM") as psC,
                tc.tile_pool(name="tpC", bufs=2, space="PSUM") as tpC,
            ):
                wo_k = []
                for kc in range(8):
                    wo1 = wC.tile([128, E], F16, name=f"wo{kc}")
                    nc.sync.dma_start(out=wo1, in_=wo_d[:, kc, :])
                    wo_k.append(wo1)
                tiles_c = _token_tiles(ntok, 256)
                pending = None  # (z2Tt, c0, cw, [(z2t, so, swd), ...])
                for ti in range(len(tiles_c) + 1):
                    if ti < len(tiles_c):
                        c0, cw = tiles_c[ti]
                        aT = cin.tile([128, 8, 256], F16, tag="aT_in")
                        nc.sync.dma_start(
                            out=aT[:, :, :cw], in_=aT_s[:, :, c0 : c0 + cw]
                        )
                        z2Tt = czt.tile([128, 8, 256], F16, tag="z2T_t")
                        subs = []
                        for so, swd in _token_tiles(cw, 128):
                            xt = cwk.tile([128, E], F32, tag="xc")
                            nc.sync.dma_start(
                                out=xt[:swd], in_=x_d[c0 + so : c0 + so + swd, :]
                            )
                            x2t = cwk.tile([128, E], F32, tag="x2c")
                            for nb in range(2):
                                ps = psC.tile([128, 512], F32, tag="mmC")
                                for kc in range(8):
                                    nc.tensor.matmul(
                                        ps[:swd], aT[:, kc, so : so + swd],
                                        wo_k[kc][:, nb * 512 : (nb + 1) * 512],
                                        start=(kc == 0), stop=(kc == 7),
                                    )
                                nc.vector.tensor_add(
                                    x2t[:swd, nb * 512 : (nb + 1) * 512], ps[:swd],
                                    xt[:swd, nb * 512 : (nb + 1) * 512],
                                )
                            nc.vector.tensor_add(x2t[:swd], x2t[:swd], bo_b[:swd])
                            nc.sync.dma_start(
                                out=x2_s[c0 + so : c0 + so + swd, :], in_=x2t[:swd]
                            )
                            z2t = cwk.tile([128, E], F16, tag="z2c", bufs=5)
                            layer_norm(cwk, x2t, swd, z2t)
                            subs.append((z2t, so, swd))
                        cur = (z2Tt, c0, cw, subs)
                    else:
                        cur = None
                    if pending is not None:
                        pz2Tt, pc0, pcw, psubs = pending
                        for z2t, so, swd in psubs:
                            for ecc in range(8):
                                tp = tpC.tile([128, 128], F16)
                                nc.tensor.transpose(
                                    tp[:, :swd],
                                    z2t[:swd, ecc * 128 : (ecc + 1) * 128],
                                    ident[:swd, :swd],
                                )
                                nc.vector.tensor_copy(
                                    pz2Tt[:, ecc, so : so + swd], tp[:, :swd]
                                )
                        nc.sync.dma_start(
                            out=z2T_s[:, :, pc0 : pc0 + pcw], in_=pz2Tt[:, :, :pcw]
                        )
                    pending = cur

            # ---------------- Phase D/E: FFN + residual ----------------
            with (
                tc.tile_pool(name="wDE", bufs=1) as wDE,
                tc.tile_pool(name="dh", bufs=2) as dh,
                tc.tile_pool(name="da1", bufs=1) as da1,
                tc.tile_pool(name="dy", bufs=2) as dy,
                tc.tile_pool(name="dout", bufs=2) as dout,
                tc.tile_pool(name="psD", bufs=3, space="PSUM") as psD,
                tc.tile_pool(name="tpD", bufs=2, space="PSUM") as tpD,
            ):
                w1_k = []
                for kc in range(8):
                    w11 = wDE.tile([128, 32, 128], F16, name=f"w1_{kc}")
                    nc.sync.dma_start(out=w11, in_=w1_d[:, kc, :, :])
                    w1_k.append(w11)
                w2_k = []
                for kg in range(4):
                    w21 = wDE.tile([128, 8, 8, 128], F16, name=f"w2_{kg}")
                    nc.sync.dma_start(out=w21, in_=w2_d[:, kg * 8 : (kg + 1) * 8, :, :])
                    w2_k.append(w21)
                for g0, gw in _token_tiles(ntok, 512):
                    hT = dh.tile([128, 8, 512], F16, tag="hT")
                    nc.sync.dma_start(
                        out=hT[:, :, :gw], in_=z2T_s[:, :, g0 : g0 + gw]
                    )
                    a1T = da1.tile([128, 32, 512], F16, tag="a1T")
                    for mc in range(32):
                        ps = psD.tile([128, 512], F32, tag="mmD")
                        for kc in range(8):
                            nc.tensor.matmul(
                                ps[:, :gw], w1_k[kc][:, mc, :], hT[:, kc, :gw],
                                start=(kc == 0), stop=(kc == 7),
                            )
                        nc.scalar.activation(
                            out=a1T[:, mc, :gw], in_=ps[:, :gw], func=Relu,
                            bias=b1t[:, mc : mc + 1], scale=1.0,
                        )
                    yT = dy.tile([128, 8, 512], F16, tag="yT")
                    for mc2 in range(8):
                        ps = psD.tile([128, 512], F32, tag="mmD")
                        for kc2 in range(32):
                            nc.tensor.matmul(
                                ps[:, :gw], w2_k[kc2 // 8][:, kc2 % 8, mc2, :], a1T[:, kc2, :gw],
                                start=(kc2 == 0), stop=(kc2 == 31),
                            )
                        nc.scalar.activation(
                            out=yT[:, mc2, :gw], in_=ps[:, :gw], func=Identity,
                            bias=b2t[:, mc2 : mc2 + 1], scale=1.0,
                        )
                    for so, swd in _token_tiles(gw, 128):
                        x2t = dout.tile([128, E], F32, tag="x2d")
                        nc.sync.dma_start(
                            out=x2t[:swd], in_=x2_s[g0 + so : g0 + so + swd, :]
                        )
                        pst = tpD.tile([128, 1024], F16)
                        for mc2 in range(8):
                            nc.tensor.transpose(
                                pst[:swd, mc2 * 128 : (mc2 + 1) * 128],
                                yT[:, mc2, so : so + swd],
                                ident[:, :],
                            )
                        nc.vector.tensor_add(x2t[:swd], pst[:swd], x2t[:swd])
                        nc.sync.dma_start(
                            out=out_d[g0 + so : g0 + so + swd, :], in_=x2t[:swd]
                        )
    return nc


def prep_weights(inputs):
    """Host-side weight preparation (fp16 casts, LN folding, layouts)."""
    f32 = np.float32
    Wq = np.asarray(inputs["Wq"], f32)
    Wk = np.asarray(inputs["Wk"], f32)
    Wv = np.asarray(inputs["Wv"], f32)
    Wo = np.asarray(inputs["Wo"], f32)
    bo = np.asarray(inputs["bo"], f32)
    W1 = np.asarray(inputs["W1"], f32)
    b1 = np.asarray(inputs["b1"], f32)
    W2 = np.asarray(inputs["W2"], f32)
    b2 = np.asarray(inputs["b2"], f32)
    ln1_w = np.asarray(inputs["ln1_w"], f32)
    ln1_b = np.asarray(inputs["ln1_b"], f32)
    ln2_w = np.asarray(inputs["ln2_w"], f32)
    ln2_b = np.asarray(inputs["ln2_b"], f32)

    assert np.all(ln1_b == 0.0) and np.all(ln2_b == 0.0), (
        "nonzero LN bias not supported by this kernel build"
    )

    # h = z*w + b with b==0  ->  fold w into the next matmul's weights (exact)
    Wq_all = np.transpose(Wq, (1, 0, 2)).reshape(E, E) * ln1_w[:, None]
    Wk_all = np.transpose(Wk, (1, 0, 2)).reshape(E, E) * ln1_w[:, None]
    Wv_all = np.transpose(Wv, (1, 0, 2)).reshape(E, E) * ln1_w[:, None]
    W1f = W1 * ln2_w[:, None]

    wqk = np.concatenate([Wq_all, Wk_all], axis=1)  # [E, 2048]
    wqk = wqk.reshape(8, 128, 16, 128).transpose(1, 0, 2, 3)
    wv = Wv_all.reshape(8, 128, E).transpose(1, 0, 2)
    wo = Wo.reshape(8, 128, E).transpose(1, 0, 2)
    w1 = W1f.reshape(8, 128, 32, 128).transpose(1, 0, 2, 3)
    w2 = W2.reshape(32, 128, 8, 128).transpose(1, 0, 2, 3)

    r = np.arange(128)
    maskT = np.where(r[:, None] <= r[None, :], 0.0, -1e9).astype(f32)

    return {
        "wqk": np.ascontiguousarray(wqk, np.float16),
        "wv": np.ascontiguousarray(wv, np.float16),
        "wo": np.ascontiguousarray(wo, np.float16),
        "w1": np.ascontiguousarray(w1, np.float16),
        "w2": np.ascontiguousarray(w2, np.float16),
        "bo_bcast": np.ascontiguousarray(np.tile(bo, (128, 1)), f32),
        "b1t": np.ascontiguousarray(b1.reshape(32, 128).T, f32),
        "b2t": np.ascontiguousarray(b2.reshape(8, 128).T, f32),
        "maskT": maskT,
        "ident16": np.eye(128, dtype=np.float16),
    }


_prog_cache = {}


def _get_program(b_loc):
    if b_loc not in _prog_cache:
        _prog_cache[b_loc] = build_program(b_loc)
    return _prog_cache[b_loc]


def kernel(**inputs) -> np.ndarray:
    from concourse.bass_utils import run_bass_kernel_spmd

    x = np.asarray(inputs["x"], np.float32)  # [B, T, E]
    wmap = prep_weights(inputs)
    nc = _get_program(B_LOC)
    in_maps = []
    for c in range(NCORES):
        m = dict(wmap)
        m["x"] = np.ascontiguousarray(
            x[c * B_LOC : (c + 1) * B_LOC].reshape(B_LOC * T, E)
        )
        in_maps.append(m)
    res = run_bass_kernel_spmd(nc, in_maps, core_ids=list(range(NCORES)))
    out = np.stack([res.results[c]["out"] for c in range(NCORES)])
    return out.reshape(B, T, E).astype(np.float32)
